# revision 1
# baseline (speedup 1.0000x reference)
"""Trainium2 Bass kernel: sparse multi-head 3x3x3 voxel conv (gnn message passing).

Self-tap (k=13) chunks use plain HWDGE DMAs (contiguous per-core shard) and
are interleaved evenly among the [128,1]-offset indirect-gather chunks so the
gpsimd queue never idles; fold is count-sorted CCE-add rounds.
"""

import sys
from contextlib import ExitStack

for p in ("/opt/trn_rl_repo", "/root/.axon_site/_ro/trn_rl_repo"):
    if p not in sys.path:
        sys.path.insert(0, p)

import numpy as np
import ml_dtypes

import concourse.tile as tile
from concourse import bass, bacc, mybir
from concourse.bass import IndirectOffsetOnAxis
from concourse.masks import make_identity

BF16 = ml_dtypes.bfloat16
C = 64
CH = 16
NH = 4
KVOL = 27
PAD_OFF = 5_000_000


def cdiv(a, b):
    return (a + b - 1) // b


def host_prep(feats, weight, kernel_map, n_cores, Q=32):
    feats = np.asarray(feats)
    weight = np.asarray(weight)
    kernel_map = np.asarray(kernel_map)
    N = feats.shape[0]
    S = N // n_cores
    ZERO_ROW = N

    table = np.zeros((N + 1, C), dtype=BF16)
    table[:N] = feats.astype(BF16)

    w_sb = np.zeros((128, KVOL * C), dtype=BF16)
    for k in range(KVOL):
        blk = np.zeros((C, C), np.float32)
        for h in range(NH):
            blk[h * CH:(h + 1) * CH, h * CH:(h + 1) * CH] = weight[k, h]
        w_sb[:C, k * C:(k + 1) * C] = blk.astype(BF16)
        w_sb[C:, k * C:(k + 1) * C] = w_sb[:C, k * C:(k + 1) * C]

    core_runs = []
    for c in range(n_cores):
        km = kernel_map[:, c * S:(c + 1) * S]
        runs = []
        for k in range(KVOL):
            m = km[k] >= 0
            runs.append((np.nonzero(m)[0].astype(np.int64),
                         km[k][m].astype(np.int64)))
        core_runs.append(runs)

    n_chunks_k = [max(cdiv(len(core_runs[c][k][0]), 128)
                      for c in range(n_cores)) for k in range(KVOL)]
    chunk_k = []
    chunk_start_k = []
    for k in range(KVOL):
        chunk_start_k.append(len(chunk_k))
        chunk_k.extend([k] * n_chunks_k[k])
    NCH_REAL = len(chunk_k)
    # pad chunk count to multiple of Q (H-batching)
    NCH = cdiv(len(chunk_k), Q) * Q
    SELF_START = chunk_start_k[13]
    N13 = n_chunks_k[13]
    # interleave self chunks (plain-DMA, no gpsimd work) evenly among
    # indirect chunks so the gpsimd gather stream never idles
    selfs = list(range(SELF_START, SELF_START + N13))
    nonself = [c for c in range(NCH_REAL) if not (SELF_START <= c < SELF_START + N13)]
    new_order = []
    acc_f = 0.0
    step = len(selfs) / max(len(nonself), 1)
    si = 0
    for c in nonself:
        new_order.append(c)
        acc_f += step
        while acc_f >= 1.0 and si < len(selfs):
            new_order.append(selfs[si]); si += 1; acc_f -= 1.0
    while si < len(selfs):
        new_order.append(selfs[si]); si += 1
    assert len(new_order) == NCH_REAL
    newpos_of = np.empty(NCH_REAL, np.int64)
    newpos_of[np.array(new_order)] = np.arange(NCH_REAL)
    chunk_k = [chunk_k[c] for c in new_order] + [0] * (NCH - NCH_REAL)
    # per-new-position: row offset into self_feats (or -1 for indirect chunks)
    self_row0 = [-1] * NCH
    for j, c in enumerate(selfs):
        self_row0[int(newpos_of[c])] = j * 128
    n_slots = NCH * 128
    NGRP = NCH // Q  # H-write groups

    # fold tiling: 128-dest tiles, count-sorted; R_t = global max per tile
    n_tiles = cdiv(S, 128)
    S_pad = n_tiles * 128
    core_counts_sorted = []
    core_orders = []
    for c in range(n_cores):
        counts = np.zeros(S, np.int64)
        for k in range(KVOL):
            counts[core_runs[c][k][0]] += 1
        order = np.argsort(-counts, kind="stable")
        core_orders.append(order)
        core_counts_sorted.append(counts[order])
    R_t = []
    for t in range(n_tiles):
        r = 0
        for c in range(n_cores):
            cs = core_counts_sorted[c]
            if t * 128 < len(cs):
                r = max(r, int(cs[t * 128]))
        R_t.append(r)
    col_base = np.concatenate([[0], np.cumsum(R_t)]).astype(np.int64)
    NR = int(col_base[-1])

    meta = dict(N=N, S=S, Q=Q, NCH=NCH, NGRP=NGRP, chunk_k=chunk_k,
                R_t=R_t, NR=NR, n_tiles=n_tiles, S_pad=S_pad,
                n_slots=n_slots, ZERO_ROW=ZERO_ROW,
                NCH_REAL=NCH_REAL, N13=N13, self_row0=self_row0)

    # h_row(s): slot s = c*128+p; group g = c//Q, q = c%Q
    # H dram row = g*128*Q + p*Q + q  (per-group partition-major, 1 desc/part)
    def h_row_of(s):
        cc = s // 128
        p = s % 128
        g = cc // Q
        q = cc % Q
        return g * 128 * Q + p * Q + q

    in_maps = []
    perms = []
    for c in range(n_cores):
        runs = core_runs[c]
        order = core_orders[c]
        rank = np.empty(S, np.int64)
        rank[order] = np.arange(S)

        gather_rows = np.full(n_slots, ZERO_ROW, np.int64)
        all_dest = []
        all_s = []
        for k in range(KVOL):
            dests, srcs = runs[k]
            L = len(dests)
            if L == 0:
                continue
            j = np.arange(L)
            s_ids = newpos_of[chunk_start_k[k] + j // 128] * 128 + (j % 128)
            gather_rows[s_ids] = srcs
            all_dest.append(dests)
            all_s.append(s_ids)
        all_dest = np.concatenate(all_dest)
        all_hrow = h_row_of(np.concatenate(all_s))

        goffs = np.ascontiguousarray(
            gather_rows.reshape(NCH, 128).T, dtype=np.int32)  # [128, NCH]

        pr = rank[all_dest]
        o2 = np.argsort(pr, kind="stable")
        sr = pr[o2]
        sh = all_hrow[o2]
        grp_start = np.searchsorted(sr, np.arange(S))
        r_idx = np.arange(len(sr)) - grp_start[sr]
        t_of = sr // 128
        p_of = sr % 128
        col = col_base[t_of] + r_idx
        assert (r_idx < np.array(R_t)[t_of]).all()
        foffs = np.full((128, NR), PAD_OFF, np.int32)
        foffs[p_of, col] = sh
        # round 0 fully initializes real dests (self tap); pad dest slots in
        # the final partial tile stay PAD -> skipped -> stale rows dropped on host.

        self_feats = np.zeros((N13 * 128, C), dtype=BF16)
        self_feats[:S] = table[c * S:(c + 1) * S]
        in_maps.append({
            "table": table,
            "w_sb": w_sb,
            "goffs": goffs,
            "foffs": foffs,
            "self_feats": self_feats,
        })
        perms.append(order)

    return in_maps, perms, meta


def build_program(n_cores, meta):
    Q, NCH, NGRP = meta["Q"], meta["NCH"], meta["NGRP"]
    chunk_k, R_t = meta["chunk_k"], meta["R_t"]
    n_tiles, S_pad, N = meta["n_tiles"], meta["S_pad"], meta["N"]
    n_H_rows = meta["n_slots"]

    nc = bacc.Bacc("TRN2", target_bir_lowering=False, debug=False,
                   num_devices=n_cores)

    table = nc.dram_tensor("table", [N + 1, C], mybir.dt.bfloat16,
                           kind="ExternalInput").ap()
    w_in = nc.dram_tensor("w_sb", [128, KVOL * C], mybir.dt.bfloat16,
                          kind="ExternalInput").ap()
    goffs = nc.dram_tensor("goffs", [128, NCH], mybir.dt.int32,
                           kind="ExternalInput").ap()
    foffs = nc.dram_tensor("foffs", [128, meta["NR"]], mybir.dt.int32,
                           kind="ExternalInput").ap()
    self_in = nc.dram_tensor("self_feats", [meta["N13"] * 128, C],
                             mybir.dt.bfloat16, kind="ExternalInput").ap()
    out = nc.dram_tensor("out", [S_pad, C], mybir.dt.float32,
                         kind="ExternalOutput").ap()

    with tile.TileContext(nc) as tc, ExitStack() as ctx:
        dram = ctx.enter_context(tc.tile_pool(name="dram", bufs=1, space="DRAM"))
        h_dram = dram.tile([n_H_rows, C], mybir.dt.bfloat16)

        wpool = ctx.enter_context(tc.tile_pool(name="w", bufs=1))
        w_t = wpool.tile([128, KVOL * C], mybir.dt.bfloat16)
        nc.sync.dma_start(out=w_t[:], in_=w_in[:])
        ident = wpool.tile([128, 128], mybir.dt.bfloat16)
        make_identity(nc, ident[:])
        gof = wpool.tile([128, NCH], mybir.dt.int32)
        nc.sync.dma_start(out=gof[:], in_=goffs[:])
        fof = wpool.tile([128, meta["NR"]], mybir.dt.int32)
        nc.sync.dma_start(out=fof[:], in_=foffs[:])

        gp = ctx.enter_context(tc.tile_pool(name="G", bufs=8))
        xp = ctx.enter_context(tc.tile_pool(name="X", bufs=8))
        hp = ctx.enter_context(tc.tile_pool(name="H", bufs=3))
        psx = ctx.enter_context(tc.tile_pool(name="psx", bufs=4, space="PSUM"))
        psh = ctx.enter_context(tc.tile_pool(name="psh", bufs=4, space="PSUM"))

        self_row0 = meta["self_row0"]
        NCH_REAL = meta["NCH_REAL"]
        for g in range(NGRP):
            h_t = hp.tile([128, Q * C], mybir.dt.bfloat16)
            for qi in range(Q):
                cid = g * Q + qi
                if cid >= NCH_REAL:
                    continue  # all-pad tail chunk: H garbage, never referenced
                k = chunk_k[cid]
                g1 = gp.tile([128, C], mybir.dt.bfloat16)
                if self_row0[cid] >= 0:
                    # self tap: sources are this core's own contiguous shard
                    j0 = self_row0[cid]
                    nc.sync.dma_start(out=g1[:], in_=self_in[j0:j0 + 128, :])
                else:
                    nc.gpsimd.indirect_dma_start(
                        out=g1[:], out_offset=None,
                        in_=table[:],
                        in_offset=IndirectOffsetOnAxis(
                            ap=gof[:, cid:cid + 1], axis=0),
                    )
                x_ps = psx.tile([64, 128], mybir.dt.bfloat16)
                nc.tensor.transpose(out=x_ps[:], in_=g1[:], identity=ident[:])
                x_t = xp.tile([64, 128], mybir.dt.bfloat16)
                nc.vector.tensor_copy(out=x_t[:], in_=x_ps[:])
                h_ps = psh.tile([128, C], mybir.dt.float32)
                nc.tensor.matmul(
                    out=h_ps[:],
                    lhsT=x_t[:],
                    rhs=w_t[0:64, k * C:(k + 1) * C],
                    start=True, stop=True,
                )
                nc.scalar.activation(
                    h_t[:, qi * C:(qi + 1) * C], h_ps[:],
                    mybir.ActivationFunctionType.Copy,
                )
            nc.sync.dma_start(
                out=h_dram[g * 128 * Q:(g + 1) * 128 * Q, :].rearrange(
                    "(p q) c -> p (q c)", p=128),
                in_=h_t[:],
            )

        fop = ctx.enter_context(tc.tile_pool(name="acc", bufs=4))
        outp = ctx.enter_context(tc.tile_pool(name="outp", bufs=4))
        col = 0
        for t in range(n_tiles):
            acc = fop.tile([128, C], mybir.dt.bfloat16)
            for r in range(R_t[t]):
                nc.gpsimd.indirect_dma_start(
                    out=acc[:], out_offset=None,
                    in_=h_dram[:],
                    in_offset=IndirectOffsetOnAxis(
                        ap=fof[:, col:col + 1], axis=0),
                    compute_op=(mybir.AluOpType.bypass if r == 0
                                else mybir.AluOpType.add),
                    bounds_check=n_H_rows - 1,
                    oob_is_err=False,
                )
                col += 1
            out_t = outp.tile([128, C], mybir.dt.float32)
            nc.vector.tensor_copy(out=out_t[:], in_=acc[:])
            nc.sync.dma_start(out=out[t * 128:(t + 1) * 128, :], in_=out_t[:])

    nc.compile()
    return nc


def assemble_output(results, perms, meta, n_cores):
    S = meta["S"]
    N = meta["N"]
    out = np.empty((N, C), np.float32)
    for c in range(n_cores):
        rows = results[c]["out"]
        out[c * S + perms[c]] = rows[:S]
    return out


N_CORES = 8
LAST_EXEC_TIME_NS = None

_CACHE = {}


def kernel(feats, weight, kernel_map):
    """Full-input entry point: shard, run on 8 NeuronCores, unshard."""
    global LAST_EXEC_TIME_NS
    import os
    from concourse import bass_utils

    feats = np.asarray(feats)
    weight = np.asarray(weight)
    kernel_map = np.asarray(kernel_map)

    in_maps, perms, meta = host_prep(feats, weight, kernel_map, N_CORES, Q=32)
    key = (meta["NCH"], meta["NR"], tuple(meta["R_t"][:4]))
    if key in _CACHE:
        nc = _CACHE[key]
    else:
        nc = build_program(N_CORES, meta)
        _CACHE[key] = nc

    trace = os.environ.get("BASS_KERNEL_TRACE", "0") == "1"
    res = bass_utils.run_bass_kernel_spmd(
        nc, in_maps, core_ids=list(range(N_CORES)), trace=trace)
    LAST_EXEC_TIME_NS = res.exec_time_ns
    return assemble_output(res.results, perms, meta, N_CORES)

